# revision 11
# baseline (speedup 1.0000x reference)
"""Trainium2 Bass kernel for the cross-attention layer:

    s   = cosine_sim(em1, em2)          # [B, N, M]
    p   = softmax(s, axis=-1)
    x   = p @ em2                       # [B, N, D]
    out = relu(concat([em1, x]) @ W.T + b)

Sharding: 8 cores, core c = 4*b + i handles batch b, query rows
[i*1024, (i+1)*1024).  em2 is replicated per batch.

v3 design (fp8 DoubleRow, host preprocessing):
  - Host precomputes input-only transforms: q^T/k^T normalized, scaled
    by 16, quantized to fp8e4 (exp scale becomes the constant 1/256);
    V and W2 raw fp8e4; and the x-independent FC term
    A = em1 @ W1.T + b as bf16 (the dominant, exactly-representable
    part of the output).  The device computes the entire attention:
    scores, softmax, P@V, x-normalization, x@W2.T, add, relu.
  - All attention matmuls are fp8 DoubleRow (K=256 per instruction).
    Per key-tile pair: 2 QK matmuls into a [128, 2, 512] PSUM pair,
    one [128, 1024]-wide Exp on ScalarE (its only op), 2 PV matmuls
    accumulating X^T directly (no transposes anywhere), and one
    all-ones-stationary matmul accumulating the softmax denominator
    (its [128, 512] output rows are all identical = free broadcast).
  - Block finish: full-partition DVE reciprocal of the rowsum bank,
    then X^T * rinv -> fp8 SBUF (FC B stationary).
  - FC B per query tile: identity-stationary matmul preloads the host
    A-term into PSUM, fp8 DR matmul accumulates x^T.T @ W2, one DVE
    max writes the f32 output tile.  GPSIMD does only DMA.
  - PSUM: 4 banks QK ping-pong + 2 banks X^T + 1 bank rowsum + 1 bank
    FC = 8.
"""

import sys

if "/opt/trn_rl_repo" not in sys.path:
    sys.path.insert(0, "/opt/trn_rl_repo")

from contextlib import ExitStack

import numpy as np

import concourse.bass as bass
import concourse.mybir as mybir
import concourse.tile as tile
from concourse import bacc
from concourse.bass_utils import run_bass_kernel_spmd
from concourse.masks import make_identity

# bass_utils imports antenv.axon_hooks when tracing is requested; this
# container's antenv lacks that submodule.  Register a stub so untraced
# runs don't crash.
try:
    import antenv.axon_hooks  # noqa: F401
except ImportError:
    import types as _types

    import antenv as _antenv

    _stub = _types.ModuleType("antenv.axon_hooks")
    _stub.get_axon_ntff_profile_hook = lambda: None
    _stub.set_axon_ntff_profile_hook = lambda h: None
    _antenv.axon_hooks = _stub
    sys.modules["antenv.axon_hooks"] = _stub

B, N, M, D = 2, 4096, 4096, 256
NSH = N // 4          # query rows per core
P = 128
NT = NSH // P         # 8 query tiles per core
MT = M // P           # 32 key tiles
NPAIR = MT // 2       # 16 key-tile pairs
OUT = 512
EPS = 1e-6
F32 = mybir.dt.float32
BF16 = mybir.dt.bfloat16
FP8 = mybir.dt.float8e4
ACTF = mybir.ActivationFunctionType
DR = mybir.MatmulPerfMode.DoubleRow
NPBF16 = mybir.dt.np(BF16)
NPFP8 = mybir.dt.np(FP8)

NBLK = 512            # query columns per block
NBLKS = NSH // NBLK   # 2
QSCALE = 16.0         # host scale on normalized q/k before fp8 quant


def build_nc():
    nc = bacc.Bacc("TRN2", target_bir_lowering=False)
    qt_d = nc.declare_dram_parameter("qt", [D, NSH], FP8, isOutput=False)
    kt_d = nc.declare_dram_parameter("kt", [D, M], FP8, isOutput=False)
    v_d = nc.declare_dram_parameter("v", [M, D], FP8, isOutput=False)
    wb_d = nc.declare_dram_parameter("wb", [D, OUT], FP8, isOutput=False)
    fa_d = nc.declare_dram_parameter("fcab", [NSH, OUT], BF16, isOutput=False)
    out_d = nc.declare_dram_parameter("out", [NSH, OUT], BF16, isOutput=True)

    with ExitStack() as ctx:
        tc = ctx.enter_context(tile.TileContext(nc))
        sb = ctx.enter_context(tc.tile_pool(name="sb", bufs=1))
        sbw = ctx.enter_context(tc.tile_pool(name="sbw", bufs=3))
        psS = ctx.enter_context(tc.tile_pool(name="psS", bufs=2, space="PSUM"))
        psX = ctx.enter_context(tc.tile_pool(name="psX", bufs=1, space="PSUM"))
        psR = ctx.enter_context(tc.tile_pool(name="psR", bufs=1, space="PSUM"))
        psF = ctx.enter_context(tc.tile_pool(name="psF", bufs=1, space="PSUM"))

        # ---- persistent SBUF ----
        qt8 = sb.tile([P, 2, NSH], FP8, tag="qt8")       # 16*qhat^T (QK moving)
        ktc = [sb.tile([P, 2, M // 4], FP8, tag=f"ktc{g}", name=f"ktc{g}")
               for g in range(4)]                        # 16*khat^T (QK stationary)
        vc = [sb.tile([P, MT // 4, D], FP8, tag=f"vc{g}", name=f"vc{g}")
              for g in range(4)]                         # raw em2 (PV stationary)
        wb = sb.tile([P, 2, OUT], FP8, tag="wb")         # W2^T fp8 (FC B moving)
        fcab = sb.tile([P, NT, OUT], BF16, tag="fcab")   # host em1@W1 + b
        hbuf = sb.tile([P, NT, OUT], BF16, tag="hbuf")   # output staging
        ident = sb.tile([P, P], BF16, tag="ident")
        ones2 = sb.tile([P, 2, P], FP8, tag="ones2")     # rowsum stationary
        xt8s = [sb.tile([P, 2, NBLK], FP8, tag=f"xt{nb}", name=f"xt{nb}")
                for nb in range(NBLKS)]
        rbcs = [sb.tile([P, NBLK], F32, tag=f"rbc{nb}", name=f"rbc{nb}")
                for nb in range(NBLKS)]

        make_identity(nc, ident)
        nc.vector.memset(ones2, 1.0)

        # ---- DMAs: 3 queues, consumer order ----
        qt_r = qt_d[:].rearrange("(do p) n -> p do n", p=P)
        kt_r = kt_d[:].rearrange("(do p) m -> p do m", p=P)
        v_r = v_d[:].rearrange("(mo p) d -> p mo d", p=P)
        wb_r = wb_d[:].rearrange("(do p) o -> p do o", p=P)
        fa_r = fa_d[:].rearrange("(no p) o -> p no o", p=P)
        out_r = out_d[:].rearrange("(no p) o -> p no o", p=P)

        def dma_ktc(g):
            ms = slice(g * (M // 4), (g + 1) * (M // 4))
            nc.sync.dma_start(ktc[g][:], kt_r[:, :, ms])

        def dma_vc(eng, g):
            mv = slice(g * (MT // 4), (g + 1) * (MT // 4))
            eng.dma_start(vc[g][:], v_r[:, mv, :])

        nc.scalar.dma_start(qt8[:], qt_r)
        dma_ktc(0)
        dma_vc(nc.sync, 0)
        dma_vc(nc.scalar, 2)
        dma_ktc(1)
        dma_vc(nc.sync, 1)
        dma_vc(nc.scalar, 3)
        dma_ktc(2)
        dma_ktc(3)
        nc.scalar.dma_start(wb[:], wb_r)
        nc.scalar.dma_start(fcab[:], fa_r)

        def fcB(nb, j, pool):
            # h[:, t] = relu(host A-term + xhat^T.T @ W2)
            t = nb * 4 + j
            if pool is psS:
                bp_ = pool.tile([P, 2, NBLK], F32, tag="sp", name=f"fcB{t}")[:, 0, :]
            else:
                bp_ = pool.tile([P, OUT], F32, tag="fc", name=f"fcB{t}")
            nc.tensor.matmul(bp_, ident[:], fcab[:, t, :], start=True, stop=False)
            js = slice(j * P, (j + 1) * P)
            nc.tensor.matmul(bp_, xt8s[nb][:, :, js], wb[:], start=False, stop=True,
                             perf_mode=DR)
            nc.vector.tensor_scalar_max(hbuf[:, t, :], bp_, 0.0)

        def out_dma(t0, t1, eng=None):
            (eng or nc.sync).dma_start(out_r[:, t0:t1, :], hbuf[:, t0:t1, :])

        def block_finish(nb, XT, rs):
            # rowsum rows are identical (all-ones stationary) -> full-
            # partition reciprocal IS the broadcast 1/rowsum.
            nc.vector.reciprocal_approx_fast(out=rbcs[nb][:], in_=rs)
            for h in range(2):
                nc.vector.tensor_mul(out=xt8s[nb][:, h, :], in0=XT[:, h, :],
                                     in1=rbcs[nb][:])

        # ---- main loop ----
        for nb in range(NBLKS):
            ncols = slice(nb * NBLK, (nb + 1) * NBLK)
            XT = psX.tile([P, 2, NBLK], F32, tag="xt", name=f"XT{nb}")
            rs = psR.tile([P, NBLK], F32, tag="rs", name=f"rs{nb}")
            pts = {}
            for i in range(NPAIR + 1):
                if i < NPAIR:
                    sp = psS.tile([P, 2, NBLK], F32, tag="sp", name=f"sp{nb}_{i}")
                    for h in range(2):
                        m = 2 * i + h
                        nc.tensor.matmul(
                            sp[:, h, :], ktc[m // 8][:, :, (m % 8) * P : (m % 8 + 1) * P],
                            qt8[:, :, ncols], start=True, stop=True, perf_mode=DR,
                        )
                    pt = sbw.tile([P, 2, NBLK], FP8, tag="pt", name=f"pt{nb}_{i}")
                    nc.scalar.activation(pt, sp, ACTF.Exp, scale=1.0 / 256.0)
                    pts[i] = pt
                if i >= 1:
                    ii = i - 1
                    pt = pts.pop(ii)
                    g, mm = ii // 4, (ii % 4) * 2
                    for j in range(2):
                        nc.tensor.matmul(
                            XT[:, j, :], vc[g][:, mm : mm + 2, j * P : (j + 1) * P],
                            pt[:], start=(ii == 0), stop=(ii == NPAIR - 1),
                            perf_mode=DR,
                        )
                    nc.tensor.matmul(
                        rs, ones2[:], pt[:], start=(ii == 0),
                        stop=(ii == NPAIR - 1), perf_mode=DR,
                    )
                # block-0 FC interleaved into block-1's loop (PE slack)
                if nb == 1:
                    if i in (3, 5, 7, 9):
                        fcB(0, (i - 3) // 2, psF)
                        if i == 5:
                            out_dma(0, 2)
                        elif i == 9:
                            out_dma(2, 4)
            block_finish(nb, XT, rs)

        # tail: FC for block 1 (psums ride in the now-free psS slots);
        # per-tile output DMA alternating across both HW queues
        for j in range(4):
            fcB(1, j, psS)
            out_dma(4 + j, 5 + j, nc.sync if j % 2 == 0 else nc.scalar)

    nc.compile()
    return nc


_NC = None


def _get_nc():
    global _NC
    if _NC is None:
        _NC = build_nc()
    return _NC


def _prep_inputs(inputs):
    em1 = np.asarray(inputs["em1"], dtype=np.float32)
    em2 = np.asarray(inputs["em2"], dtype=np.float32)
    W = np.asarray(inputs["W"], dtype=np.float32)
    b = np.asarray(inputs["b"], dtype=np.float32)

    def norm16(x):  # QSCALE * x / sqrt(max(|x|^2, eps))
        n2 = np.sum(x * x, axis=-1, keepdims=True)
        return x * (QSCALE / np.sqrt(np.maximum(n2, EPS)))

    wb = np.ascontiguousarray(W.T[D : 2 * D]).astype(NPFP8)   # [D, OUT] fp8
    kts = [np.ascontiguousarray(norm16(em2[bi]).T).astype(NPFP8) for bi in range(B)]
    vs = [em2[bi].astype(NPFP8) for bi in range(B)]
    q16 = [norm16(em1[bi]) for bi in range(B)]
    # x-independent FC term, exact in f32 then rounded to bf16
    fcabs = [(em1[bi] @ W.T[0:D] + b).astype(NPBF16) for bi in range(B)]
    in_maps = []
    for c in range(8):
        bi, qi = c // 4, c % 4
        cs = slice(qi * NSH, (qi + 1) * NSH)
        in_maps.append(
            {
                "qt": np.ascontiguousarray(q16[bi][cs].T).astype(NPFP8),
                "kt": kts[bi],
                "v": vs[bi],
                "wb": wb,
                "fcab": np.ascontiguousarray(fcabs[bi][cs]),
            }
        )
    return in_maps


def _run(inputs, trace=False):
    in_maps = _prep_inputs(inputs)
    res = run_bass_kernel_spmd(_get_nc(), in_maps, core_ids=list(range(8)), trace=trace)
    out = np.empty((B, N, OUT), dtype=np.float32)
    for c in range(8):
        bi, qi = c // 4, c % 4
        out[bi, qi * NSH : (qi + 1) * NSH] = res.results[c]["out"].astype(np.float32)
    return out, res


def kernel(**inputs) -> np.ndarray:
    out, _ = _run(inputs, trace=False)
    return out


# revision 14
# speedup vs baseline: 1.2384x; 1.2384x over previous
"""Trainium2 Bass kernel for the cross-attention layer:

    s   = cosine_sim(em1, em2)          # [B, N, M]
    p   = softmax(s, axis=-1)
    x   = p @ em2                       # [B, N, D]
    out = relu(concat([em1, x]) @ W.T + b)

Sharding: 8 cores, core c = 4*b + i handles batch b, query rows
[i*1024, (i+1)*1024).  em2 is replicated per batch.

v3 design (fp8 DoubleRow, host preprocessing):
  - Host precomputes input-only transforms: q^T/k^T normalized, scaled
    by 16, quantized to fp8e4 (exp scale becomes the constant 1/256);
    V and W2 raw fp8e4; and the x-independent FC term
    A = em1 @ W1.T + b as bf16 (the dominant, exactly-representable
    part of the output).  The device computes the entire attention:
    scores, softmax, P@V, x-normalization, x@W2.T, add, relu.
  - All attention matmuls are fp8 DoubleRow (K=256 per instruction).
    Per key-tile pair: 2 QK matmuls into a [128, 2, 512] PSUM pair,
    one [128, 1024]-wide Exp on ScalarE (its only op), 2 PV matmuls
    accumulating X^T directly (no transposes anywhere), and one
    all-ones-stationary matmul accumulating the softmax denominator
    (its [128, 512] output rows are all identical = free broadcast).
  - Block finish: full-partition DVE reciprocal of the rowsum bank,
    then X^T * rinv -> fp8 SBUF (FC B stationary).
  - FC B per query tile: identity-stationary matmul preloads the host
    A-term into PSUM, fp8 DR matmul accumulates x^T.T @ W2, one DVE
    max writes the f32 output tile.  GPSIMD does only DMA.
  - PSUM: 4 banks QK ping-pong + 2 banks X^T + 1 bank rowsum + 1 bank
    FC = 8.
"""

import sys

if "/opt/trn_rl_repo" not in sys.path:
    sys.path.insert(0, "/opt/trn_rl_repo")

from contextlib import ExitStack

import numpy as np

import concourse.bass as bass
import concourse.mybir as mybir
import concourse.tile as tile
from concourse import bacc
from concourse.bass_utils import run_bass_kernel_spmd
from concourse.masks import make_identity

# bass_utils imports antenv.axon_hooks when tracing is requested; this
# container's antenv lacks that submodule.  Register a stub so untraced
# runs don't crash.
try:
    import antenv.axon_hooks  # noqa: F401
except ImportError:
    import types as _types

    import antenv as _antenv

    _stub = _types.ModuleType("antenv.axon_hooks")
    _stub.get_axon_ntff_profile_hook = lambda: None
    _stub.set_axon_ntff_profile_hook = lambda h: None
    _antenv.axon_hooks = _stub
    sys.modules["antenv.axon_hooks"] = _stub

B, N, M, D = 2, 4096, 4096, 256
NSH = N // 4          # query rows per core
P = 128
NT = NSH // P         # 8 query tiles per core
MT = M // P           # 32 key tiles
NPAIR = MT // 2       # 16 key-tile pairs
OUT = 512
EPS = 1e-6
F32 = mybir.dt.float32
BF16 = mybir.dt.bfloat16
FP8 = mybir.dt.float8e4
ACTF = mybir.ActivationFunctionType
DR = mybir.MatmulPerfMode.DoubleRow
NPBF16 = mybir.dt.np(BF16)
NPFP8 = mybir.dt.np(FP8)

NBLK = 512            # query columns per block
NBLKS = NSH // NBLK   # 2
QSCALE = 16.0         # host scale on normalized q/k before fp8 quant


def build_nc():
    nc = bacc.Bacc("TRN2", target_bir_lowering=False)
    # all inputs arrive pre-swizzled by the host into their exact SBUF
    # image [128, bytes] so every DMA is fully contiguous per partition
    qt_d = nc.declare_dram_parameter("qt", [P, 2 * NSH], FP8, isOutput=False)
    kt_ds = [nc.declare_dram_parameter(f"kt{g}", [P, 2 * (M // 4)], FP8,
                                       isOutput=False) for g in range(4)]
    v_ds = [nc.declare_dram_parameter(f"v{g}", [P, (MT // 4) * D], FP8,
                                      isOutput=False) for g in range(4)]
    wb_d = nc.declare_dram_parameter("wb", [P, 2 * OUT], FP8, isOutput=False)
    fa_d = nc.declare_dram_parameter("fcab", [P, NT * OUT], BF16, isOutput=False)
    out_d = nc.declare_dram_parameter("out", [P, NT * OUT], BF16, isOutput=True)

    with ExitStack() as ctx:
        tc = ctx.enter_context(tile.TileContext(nc))
        sb = ctx.enter_context(tc.tile_pool(name="sb", bufs=1))
        sbw = ctx.enter_context(tc.tile_pool(name="sbw", bufs=3))
        psS = ctx.enter_context(tc.tile_pool(name="psS", bufs=2, space="PSUM"))
        psX = ctx.enter_context(tc.tile_pool(name="psX", bufs=1, space="PSUM"))
        psR = ctx.enter_context(tc.tile_pool(name="psR", bufs=1, space="PSUM"))
        psF = ctx.enter_context(tc.tile_pool(name="psF", bufs=1, space="PSUM"))

        # ---- persistent SBUF ----
        qt8 = sb.tile([P, 2, NSH], FP8, tag="qt8")       # 16*qhat^T (QK moving)
        ktc = [sb.tile([P, 2, M // 4], FP8, tag=f"ktc{g}", name=f"ktc{g}")
               for g in range(4)]                        # 16*khat^T (QK stationary)
        vc = [sb.tile([P, MT // 4, D], FP8, tag=f"vc{g}", name=f"vc{g}")
              for g in range(4)]                         # raw em2 (PV stationary)
        wb = sb.tile([P, 2, OUT], FP8, tag="wb")         # W2^T fp8 (FC B moving)
        fcab = sb.tile([P, NT, OUT], BF16, tag="fcab")   # host em1@W1 + b
        hbuf = sb.tile([P, NT, OUT], BF16, tag="hbuf")   # output staging
        ident = sb.tile([P, P], BF16, tag="ident")
        ones2 = sb.tile([P, 2, P], FP8, tag="ones2")     # rowsum stationary
        xt8s = [sb.tile([P, 2, NBLK], FP8, tag=f"xt{nb}", name=f"xt{nb}")
                for nb in range(NBLKS)]
        rbcs = [sb.tile([P, NBLK], F32, tag=f"rbc{nb}", name=f"rbc{nb}")
                for nb in range(NBLKS)]

        # ---- DMAs: 3 queues, fully-contiguous transfers, consumer order
        out_r = out_d[:].rearrange("p (no o) -> p no o", o=OUT)

        nc.scalar.dma_start(qt8[:], qt_d[:].rearrange("p (do n) -> p do n", do=2))
        nc.gpsimd.dma_start(vc[0][:], v_ds[0][:].rearrange("p (mo d) -> p mo d", d=D))
        nc.sync.dma_start(ktc[0][:], kt_ds[0][:].rearrange("p (do m) -> p do m", do=2))
        nc.gpsimd.dma_start(vc[1][:], v_ds[1][:].rearrange("p (mo d) -> p mo d", d=D))
        nc.scalar.dma_start(vc[2][:], v_ds[2][:].rearrange("p (mo d) -> p mo d", d=D))
        nc.sync.dma_start(ktc[1][:], kt_ds[1][:].rearrange("p (do m) -> p do m", do=2))
        nc.scalar.dma_start(vc[3][:], v_ds[3][:].rearrange("p (mo d) -> p mo d", d=D))
        nc.sync.dma_start(ktc[2][:], kt_ds[2][:].rearrange("p (do m) -> p do m", do=2))
        nc.sync.dma_start(ktc[3][:], kt_ds[3][:].rearrange("p (do m) -> p do m", do=2))
        nc.scalar.dma_start(wb[:], wb_d[:].rearrange("p (do o) -> p do o", do=2))
        nc.gpsimd.dma_start(fcab[:], fa_d[:].rearrange("p (no o) -> p no o", o=OUT))

        make_identity(nc, ident)
        nc.vector.memset(ones2, 1.0)

        def fcB(nb, j, pool):
            # h[:, t] = relu(host A-term + xhat^T.T @ W2)
            t = nb * 4 + j
            if pool is psS:
                bp_ = pool.tile([P, 2, NBLK], F32, tag="sp", name=f"fcB{t}")[:, 0, :]
            else:
                bp_ = pool.tile([P, OUT], F32, tag="fc", name=f"fcB{t}")
            nc.tensor.matmul(bp_, ident[:], fcab[:, t, :], start=True, stop=False)
            js = slice(j * P, (j + 1) * P)
            nc.tensor.matmul(bp_, xt8s[nb][:, :, js], wb[:], start=False, stop=True,
                             perf_mode=DR)
            nc.vector.tensor_scalar_max(hbuf[:, t, :], bp_, 0.0)

        def out_dma(t0, t1, eng=None):
            (eng or nc.sync).dma_start(out_r[:, t0:t1, :], hbuf[:, t0:t1, :])

        def block_finish(nb, XT, rs):
            # rowsum rows are identical (all-ones stationary) -> full-
            # partition reciprocal IS the broadcast 1/rowsum.
            nc.vector.reciprocal_approx_fast(out=rbcs[nb][:], in_=rs)
            for h in range(2):
                nc.vector.tensor_mul(out=xt8s[nb][:, h, :], in0=XT[:, h, :],
                                     in1=rbcs[nb][:])

        # ---- main loop ----
        for nb in range(NBLKS):
            ncols = slice(nb * NBLK, (nb + 1) * NBLK)
            XT = psX.tile([P, 2, NBLK], F32, tag="xt", name=f"XT{nb}")
            rs = psR.tile([P, NBLK], F32, tag="rs", name=f"rs{nb}")
            pts = {}
            for i in range(NPAIR + 1):
                if i < NPAIR:
                    sp = psS.tile([P, 2, NBLK], F32, tag="sp", name=f"sp{nb}_{i}")
                    for h in range(2):
                        m = 2 * i + h
                        nc.tensor.matmul(
                            sp[:, h, :], ktc[m // 8][:, :, (m % 8) * P : (m % 8 + 1) * P],
                            qt8[:, :, ncols], start=True, stop=True, perf_mode=DR,
                        )
                    pt = sbw.tile([P, 2, NBLK], FP8, tag="pt", name=f"pt{nb}_{i}")
                    nc.scalar.activation(pt, sp, ACTF.Exp, scale=1.0 / 256.0)
                    pts[i] = pt
                if i >= 1:
                    ii = i - 1
                    pt = pts.pop(ii)
                    g, mm = ii // 4, (ii % 4) * 2
                    for j in range(2):
                        nc.tensor.matmul(
                            XT[:, j, :], vc[g][:, mm : mm + 2, j * P : (j + 1) * P],
                            pt[:], start=(ii == 0), stop=(ii == NPAIR - 1),
                            perf_mode=DR,
                        )
                    nc.tensor.matmul(
                        rs, ones2[:], pt[:], start=(ii == 0),
                        stop=(ii == NPAIR - 1), perf_mode=DR,
                    )
                # block-0 FC interleaved into block-1's loop (PE slack)
                if nb == 1:
                    if i in (3, 5, 7, 9):
                        fcB(0, (i - 3) // 2, psF)
                        if i == 5:
                            out_dma(0, 2)
                        elif i == 9:
                            out_dma(2, 4)
            block_finish(nb, XT, rs)

        # tail: FC for block 1 (psums ride in the now-free psS slots);
        # per-tile output DMA alternating across both HW queues
        for j in range(4):
            fcB(1, j, psS)
            out_dma(4 + j, 5 + j, nc.sync if j % 2 == 0 else nc.scalar)

    nc.compile()
    return nc


_NC = None


def _get_nc():
    global _NC
    if _NC is None:
        _NC = build_nc()
    return _NC


def _prep_inputs(inputs):
    em1 = np.asarray(inputs["em1"], dtype=np.float32)
    em2 = np.asarray(inputs["em2"], dtype=np.float32)
    W = np.asarray(inputs["W"], dtype=np.float32)
    b = np.asarray(inputs["b"], dtype=np.float32)

    def norm16(x):  # QSCALE * x / sqrt(max(|x|^2, eps))
        n2 = np.sum(x * x, axis=-1, keepdims=True)
        return x * (QSCALE / np.sqrt(np.maximum(n2, EPS)))

    def sw_dhalf(a):  # [D, X] -> [128, 2*X] (partition = d % 128)
        Dd, X = a.shape
        return np.ascontiguousarray(
            a.reshape(2, P, X).transpose(1, 0, 2).reshape(P, 2 * X))

    def sw_rows(a):  # [R, X] -> [128, (R//128)*X] (partition = r % 128)
        R, X = a.shape
        return np.ascontiguousarray(
            a.reshape(R // P, P, X).transpose(1, 0, 2).reshape(P, -1))

    wb = sw_dhalf(W.T[D : 2 * D].astype(NPFP8))
    kts = []
    for bi in range(B):
        ktT = norm16(em2[bi]).T.astype(NPFP8)          # [D, M]
        kts.append([sw_dhalf(ktT[:, g * (M // 4) : (g + 1) * (M // 4)])
                    for g in range(4)])
    vs = [sw_rows(em2[bi].astype(NPFP8)) for bi in range(B)]
    q16 = [norm16(em1[bi]) for bi in range(B)]
    # x-independent FC term, exact in f32 then rounded to bf16
    fcabs = [(em1[bi] @ W.T[0:D] + b).astype(NPBF16) for bi in range(B)]
    in_maps = []
    for c in range(8):
        bi, qi = c // 4, c % 4
        cs = slice(qi * NSH, (qi + 1) * NSH)
        m = {
            "qt": sw_dhalf(q16[bi][cs].T.astype(NPFP8)),
            "wb": wb,
            "fcab": sw_rows(fcabs[bi][cs]),
        }
        for g in range(4):
            m[f"kt{g}"] = kts[bi][g]
            m[f"v{g}"] = np.ascontiguousarray(
                vs[bi][:, g * ((MT // 4) * D) : (g + 1) * ((MT // 4) * D)])
        in_maps.append(m)
    return in_maps


def _run(inputs, trace=False):
    in_maps = _prep_inputs(inputs)
    res = run_bass_kernel_spmd(_get_nc(), in_maps, core_ids=list(range(8)), trace=trace)
    out = np.empty((B, N, OUT), dtype=np.float32)
    for c in range(8):
        bi, qi = c // 4, c % 4
        o = res.results[c]["out"].astype(np.float32)          # [128, NT*OUT]
        o = o.reshape(P, NT, OUT).transpose(1, 0, 2).reshape(NSH, OUT)
        out[bi, qi * NSH : (qi + 1) * NSH] = o
    return out, res


def kernel(**inputs) -> np.ndarray:
    out, _ = _run(inputs, trace=False)
    return out
